# revision 8
# baseline (speedup 1.0000x reference)
"""GCN layer (GCNConv + BatchNorm + ReLU) as an 8-core Trainium2 Bass kernel.

Math (reference):
    xw   = x @ W.T
    deg  = in-degree (incl. self loop) at targets
    out0[t] = sum_{e: col_e = t} dis[row_e] * dis[t] * xw[row_e]   (+ bias)
    out  = relu(batchnorm(out0))
  with dis = 1/sqrt(deg).  The bias cancels inside training-mode BN, so it is
  skipped.  Factorization: xs[n] = dis[n] * xw[n]; out0[t] = dis[t] * sum xs[row_e].

Distribution: target nodes sharded over 8 cores.  Host partitions + sorts the
edges by target and pads each 128-target group's edge list to 128-edge tiles
(separately for sources < SPLIT and >= SPLIT, because dma_gather indices are
int16).  Device: per-shard xs = dis*(x@W.T) -> AllGather full bf16 xs table ->
dma_gather per-edge rows -> segment-sum via one-hot-mask matmuls accumulated
in PSUM -> BN stats partial sums -> 1KB AllReduce -> scale/shift + ReLU.
"""

import sys

for _p in ("/opt/trn_rl_repo",):
    if _p not in sys.path:
        sys.path.insert(0, _p)

import numpy as np
import ml_dtypes

BF16_NP = ml_dtypes.bfloat16


def _cdiv(a, b):
    return -(-a // b)


# ---------------------------------------------------------------------------
# Host-side preparation: shard + sort edges, build padded tile structure.
# ---------------------------------------------------------------------------


def host_prep(x, edge_index, W, gamma, beta, n_cores, split, groups_per_chunk=4):
    """Returns (struct, in_maps).  struct holds the Python constants shared by
    every core (tile counts etc); in_maps the per-core input tensors."""
    N, C = x.shape
    assert N % n_cores == 0
    NSH = N // n_cores
    NG = _cdiv(NSH, 128)

    row = np.asarray(edge_index[0], dtype=np.int64)
    col = np.asarray(edge_index[1], dtype=np.int64)
    loops = np.arange(N, dtype=np.int64)
    row = np.concatenate([row, loops])
    col = np.concatenate([col, loops])

    per_core = []
    nA = np.zeros((n_cores, NG), dtype=np.int64)
    nB = np.zeros((n_cores, NG), dtype=np.int64)
    for c in range(n_cores):
        sel = (col >= c * NSH) & (col < (c + 1) * NSH)
        rc, cc = row[sel], col[sel] - c * NSH
        order = np.argsort(cc, kind="stable")
        rc, cc = rc[order], cc[order]
        # per-local-node boundaries (degree = ends - starts, device computes it)
        starts = np.searchsorted(cc, np.arange(NSH), side="left")
        ends = np.searchsorted(cc, np.arange(NSH) + 1, side="left")
        ga = []
        gb = []
        for g in range(NG):
            gw = min(128, NSH - g * 128)
            lo = np.searchsorted(cc, g * 128, side="left")
            hi = np.searchsorted(cc, g * 128 + gw, side="left")
            r_g, c_g = rc[lo:hi], cc[lo:hi] - g * 128
            p_g = (r_g // NSH) * (NG * 128) + (r_g % NSH)  # padded table row
            a_m = p_g < split
            ga.append((p_g[a_m], c_g[a_m]))
            gb.append((p_g[~a_m] - split, c_g[~a_m]))
            nA[c, g] = int(a_m.sum())
            nB[c, g] = int(hi - lo - nA[c, g])
        per_core.append((starts, ends, ga, gb))

    tA = [int(_cdiv(int(nA[:, g].max()), 128)) for g in range(NG)]
    tB = [int(_cdiv(int(nB[:, g].max()), 128)) for g in range(NG)]
    TA, TB = sum(tA), sum(tB)
    LA, LB = TA * 128, TB * 128

    # chunks of whole groups for the gather pipeline
    chunks = []
    g = 0
    while g < NG:
        g2 = min(NG, g + groups_per_chunk)
        chunks.append((g, g2, sum(tA[g:g2]), sum(tB[g:g2])))
        g = g2

    NGP = NG * 128  # padded shard rows in the xs table
    NPAD = n_cores * NGP

    struct = dict(
        N=N, C=C, P=n_cores, NSH=NSH, NG=NG, SPLIT=split, NGP=NGP, NPAD=NPAD,
        tA=tA, tB=tB, TA=TA, TB=TB, LA=LA, LB=LB, chunks=chunks,
        gw=[min(128, NSH - g * 128) for g in range(NG)],
    )

    def wrap_idx(arr):
        # position i -> [i % 16, i // 16], replicated over the 8 blocks of 16
        L = len(arr)
        w = arr.reshape(L // 16, 16).T.astype(np.int16)
        return np.tile(w, (8, 1))

    in_maps = []
    for c in range(n_cores):
        starts, ends, ga, gb = per_core[c]
        idxA = np.zeros(LA, dtype=np.int64)
        colA = np.full(TA * 128, 255, dtype=np.int64)
        idxB = np.zeros(LB, dtype=np.int64)
        colB = np.full(TB * 128, 255, dtype=np.int64)
        oa = ob = 0
        for g in range(NG):
            r_g, c_g = ga[g]
            idxA[oa: oa + len(r_g)] = r_g
            colA[oa: oa + len(c_g)] = c_g
            oa += tA[g] * 128
            r_g, c_g = gb[g]
            idxB[ob: ob + len(r_g)] = r_g
            colB[ob: ob + len(c_g)] = c_g
            ob += tB[g] * 128

        segs = np.zeros((128, 2 * NG), dtype=np.int32)
        s_pad = np.zeros(NG * 128, dtype=np.int64)
        e_pad = np.ones(NG * 128, dtype=np.int64)
        s_pad[:NSH] = starts
        e_pad[:NSH] = ends
        segs[:, :NG] = s_pad.reshape(NG, 128).T
        segs[:, NG:] = e_pad.reshape(NG, 128).T

        xT = np.zeros((C, NG * 128), dtype=BF16_NP)
        xT[:, :NSH] = np.asarray(x[c * NSH: (c + 1) * NSH], dtype=np.float32).T

        WT = np.ascontiguousarray(np.asarray(W, np.float32).T).astype(BF16_NP)
        iota = np.tile(np.arange(128), (128, 1)).astype(BF16_NP)
        cbf = np.concatenate([xT, WT, iota], axis=1)

        colA_w = (colA.reshape(TA, 128).T.astype(np.float32)
                  if TA else np.zeros((128, 1), np.float32))
        colB_w = (colB.reshape(TB, 128).T.astype(np.float32)
                  if TB else np.zeros((128, 1), np.float32))
        cf32 = np.concatenate([
            np.eye(128, dtype=np.float32),
            segs.view(np.float32),
            colA_w, colB_w,
            np.tile(np.asarray(gamma, np.float32).reshape(C, 1), (1, 1)),
            np.tile(np.asarray(beta, np.float32).reshape(C, 1), (1, 1)),
        ], axis=1)

        ci16 = np.concatenate([
            wrap_idx(idxA) if LA else np.zeros((128, 1), np.int16),
            wrap_idx(idxB) if LB else np.zeros((128, 1), np.int16),
        ], axis=1)

        m = {"cbf": cbf, "cf32": cf32, "ci16": ci16}
        in_maps.append(m)
    return struct, in_maps


# ---------------------------------------------------------------------------
# Device program (one SPMD program; all per-core differences live in data).
# ---------------------------------------------------------------------------


def build_program(st):
    import concourse.bass as bass
    import concourse.bacc as bacc
    import concourse.mybir as mybir
    from concourse import tile

    F32 = mybir.dt.float32
    BF16 = mybir.dt.bfloat16
    I16 = mybir.dt.int16
    I32 = mybir.dt.int32

    N, C, P = st["N"], st["C"], st["P"]
    NSH, NG, SPLIT = st["NSH"], st["NG"], st["SPLIT"]
    NGP, NPAD = st["NGP"], st["NPAD"]
    tA, tB, TA, TB = st["tA"], st["tB"], st["TA"], st["TB"]
    LA, LB = st["LA"], st["LB"]
    chunks, gw = st["chunks"], st["gw"]
    TAw, TBw = max(1, TA), max(1, TB)

    nc = bacc.Bacc(num_devices=P)

    # I/O (combined per-dtype to keep DMA-lane fan-in small)
    W_BF = NG * 128 + C + 128
    W_F32 = 128 + 2 * NG + TAw + TBw + 2
    W_I16 = max(1, LA // 16) + max(1, LB // 16)
    cbf_d = nc.dram_tensor("cbf", [128, W_BF], BF16, kind="ExternalInput")
    cf32_d = nc.dram_tensor("cf32", [128, W_F32], F32, kind="ExternalInput")
    ci16_d = nc.dram_tensor("ci16", [128, W_I16], I16, kind="ExternalInput")
    out_d = nc.dram_tensor("outT", [C, NSH], F32, kind="ExternalOutput")

    rg = [list(range(P))]

    with tile.TileContext(nc) as tc:
        with (
            tc.tile_pool(name="persist", bufs=1) as pp,
            tc.tile_pool(name="xs_sb", bufs=3) as xs_pool,
            tc.tile_pool(name="gA", bufs=2) as gA_pool,
            tc.tile_pool(name="gB", bufs=2) as gB_pool,
            tc.tile_pool(name="mask", bufs=6) as mask_pool,
            tc.tile_pool(name="scratch", bufs=2) as scr_pool,
            tc.tile_pool(name="disrow", bufs=2) as disrow_pool,
            tc.tile_pool(name="disbc", bufs=2) as disbc_pool,
            tc.tile_pool(name="psx", bufs=2, space="PSUM") as psx_pool,
            tc.tile_pool(name="psagg", bufs=2, space="PSUM") as psagg_pool,
            tc.tile_pool(name="pstr", bufs=2, space="PSUM") as pstr_pool,
            tc.tile_pool(name="psdis", bufs=2, space="PSUM") as psdis_pool,
            tc.tile_pool(name="dram", bufs=1, space="DRAM") as dram,
        ):
            # ---- resident tiles + 3 combined input loads
            cbf = pp.tile([128, W_BF], BF16)
            cf32 = pp.tile([128, W_F32], F32)
            ci16 = pp.tile([128, W_I16], I16)
            nc.sync.dma_start(cbf[:], cbf_d[:])
            nc.sync.dma_start(cf32[:], cf32_d[:])
            nc.sync.dma_start(ci16[:], ci16_d[:])

            xT = cbf[:, 0: NG * 128]
            WT = cbf[:, NG * 128: NG * 128 + C]
            iota = cbf[:, NG * 128 + C: NG * 128 + C + 128]
            ident = cf32[:, 0:128]
            segs = cf32[:, 128: 128 + 2 * NG].bitcast(I32)
            colA = cf32[:, 128 + 2 * NG: 128 + 2 * NG + TAw]
            colB = cf32[:, 128 + 2 * NG + TAw: 128 + 2 * NG + TAw + TBw]
            gamma = cf32[:, W_F32 - 2: W_F32 - 1]
            beta = cf32[:, W_F32 - 1: W_F32]
            idxA = ci16[:, 0: max(1, LA // 16)]
            idxB = ci16[:, max(1, LA // 16): W_I16]
            ones_row = pp.tile([1, 128], F32)
            nc.vector.memset(ones_row[:], 1.0)

            pre = pp.tile([C, NSH], F32)          # out0^T accumulator
            sum_cols = pp.tile([C, NG], F32)
            sumsq_cols = pp.tile([C, NG], F32)
            deg_i = pp.tile([128, NG], I32)
            deg_f = pp.tile([128, NG], F32)
            rec = pp.tile([128, NG], F32)
            dis_w = pp.tile([128, NG], F32)
            stats = pp.tile([C, 2], F32)
            stats_res = pp.tile([C, 2], F32)

            # ---- degree -> dis = sqrt(1/deg), wrapped layout [p, g]
            nc.vector.tensor_tensor(deg_i[:], segs[:, NG:2 * NG], segs[:, 0:NG],
                                    mybir.AluOpType.subtract)
            nc.vector.tensor_copy(deg_f[:], deg_i[:])
            nc.vector.reciprocal(rec[:], deg_f[:])
            nc.scalar.activation(dis_w[:], rec[:], mybir.ActivationFunctionType.Sqrt)

            # ---- xs table: per-shard xs = dis * (x @ W.T), bf16, -> DRAM, AllGather
            xs_shard = dram.tile([NGP, C], BF16)
            xs_full = dram.tile([NPAD, C], BF16, addr_space="Shared")
            xs_all = pp.tile([128, NG * C], BF16)
            for g in range(NG):
                ps = psx_pool.tile([128, C], F32)
                nc.tensor.matmul(ps[:], xT[:, g * 128: (g + 1) * 128], WT[:],
                                 start=True, stop=True)
                nc.scalar.activation(xs_all[:, g * C: (g + 1) * C], ps[:],
                                     mybir.ActivationFunctionType.Copy,
                                     scale=dis_w[:, g: g + 1])
            nc.sync.dma_start(
                xs_shard[:].rearrange("(g p) c -> p g c", p=128),
                xs_all[:].rearrange("p (g c) -> p g c", c=C))
            nc.gpsimd.collective_compute(
                "AllGather", mybir.AluOpType.bypass, replica_groups=rg,
                ins=[xs_shard[:]], outs=[xs_full[:]],
            )

            # ---- main loop: gather chunks, mask-matmul segment sums
            ofsA = ofsB = 0
            for (g0, g1, ntA, ntB) in chunks:
                gbufA = gbufB = None
                if ntA:
                    gbufA = gA_pool.tile([128, ntA * 128], BF16, tag="gA")
                    nc.gpsimd.dma_gather(
                        gbufA[:].rearrange("p (s e) -> p s e", e=128),
                        xs_full[0:SPLIT, :],
                        idxA[:, ofsA * 8: (ofsA + ntA) * 8],
                        ntA * 128, ntA * 128, C, single_packet=False,
                    )
                if ntB:
                    gbufB = gB_pool.tile([128, ntB * 128], BF16, tag="gB")
                    nc.gpsimd.dma_gather(
                        gbufB[:].rearrange("p (s e) -> p s e", e=128),
                        xs_full[SPLIT:NPAD, :],
                        idxB[:, ofsB * 8: (ofsB + ntB) * 8],
                        ntB * 128, ntB * 128, C, single_packet=False,
                    )
                la = lb = 0
                for g in range(g0, g1):
                    w = gw[g]
                    nmm = tA[g] + tB[g]
                    ps = psagg_pool.tile([128, 128], F32)
                    k = 0
                    for t in range(tA[g]):
                        mk = mask_pool.tile([128, 128], BF16)
                        nc.vector.tensor_scalar(
                            mk[:, :w], iota[:, :w],
                            colA[:, ofsA + la + t: ofsA + la + t + 1], None,
                            mybir.AluOpType.is_equal)
                        nc.tensor.matmul(
                            ps[:, :w],
                            gbufA[:, (la + t) * 128: (la + t + 1) * 128],
                            mk[:, :w], start=(k == 0), stop=(k == nmm - 1))
                        k += 1
                    for t in range(tB[g]):
                        mk = mask_pool.tile([128, 128], BF16)
                        nc.vector.tensor_scalar(
                            mk[:, :w], iota[:, :w],
                            colB[:, ofsB + lb + t: ofsB + lb + t + 1], None,
                            mybir.AluOpType.is_equal)
                        nc.tensor.matmul(
                            ps[:, :w],
                            gbufB[:, (lb + t) * 128: (lb + t + 1) * 128],
                            mk[:, :w], start=(k == 0), stop=(k == nmm - 1))
                        k += 1
                    la += tA[g]
                    lb += tB[g]

                    # epilogue: pre[:, g] = dis[t] * ps ; stats partials
                    pst = pstr_pool.tile([1, 128], F32)
                    nc.tensor.transpose(pst[:], dis_w[:, g: g + 1], ident[:])
                    drow = disrow_pool.tile([1, 128], F32)
                    nc.scalar.activation(drow[:], pst[:],
                                         mybir.ActivationFunctionType.Copy)
                    psd = psdis_pool.tile([128, 128], F32)
                    nc.tensor.matmul(psd[:, :w], ones_row[:], drow[:, :w],
                                     start=True, stop=True)
                    dis_bc = disbc_pool.tile([128, 128], F32)
                    nc.scalar.activation(dis_bc[:, :w], psd[:, :w],
                                         mybir.ActivationFunctionType.Copy)
                    pre_sl = pre[:, g * 128: g * 128 + w]
                    nc.vector.tensor_tensor(pre_sl, ps[:, :w], dis_bc[:, :w],
                                            mybir.AluOpType.mult)
                    sq = scr_pool.tile([128, 128], F32)
                    nc.scalar.activation(sq[:, :w], pre_sl,
                                         mybir.ActivationFunctionType.Square,
                                         accum_out=sumsq_cols[:, g: g + 1])
                    nc.vector.tensor_reduce(sum_cols[:, g: g + 1], pre_sl,
                                            mybir.AxisListType.X,
                                            mybir.AluOpType.add)
                ofsA += ntA
                ofsB += ntB

            # ---- BN stats AllReduce + finalize
            nc.vector.tensor_reduce(stats[:, 0:1], sum_cols[:],
                                    mybir.AxisListType.X, mybir.AluOpType.add)
            nc.vector.tensor_reduce(stats[:, 1:2], sumsq_cols[:],
                                    mybir.AxisListType.X, mybir.AluOpType.add)
            st_in = dram.tile([C, 2], F32)
            st_out = dram.tile([C, 2], F32, addr_space="Shared")
            nc.sync.dma_start(st_in[:], stats[:])
            nc.gpsimd.collective_compute(
                "AllReduce", mybir.AluOpType.add, replica_groups=rg,
                ins=[st_in[:]], outs=[st_out[:]],
            )
            nc.sync.dma_start(stats_res[:], st_out[:])

            mean = pp.tile([C, 1], F32)
            ex2 = pp.tile([C, 1], F32)
            var = pp.tile([C, 1], F32)
            std = pp.tile([C, 1], F32)
            rstd = pp.tile([C, 1], F32)
            scl = pp.tile([C, 1], F32)
            shf = pp.tile([C, 1], F32)
            tmp = pp.tile([C, 1], F32)
            inv_n = 1.0 / float(N)
            nc.vector.tensor_scalar_mul(mean[:], stats_res[:, 0:1], inv_n)
            nc.vector.tensor_scalar_mul(ex2[:], stats_res[:, 1:2], inv_n)
            nc.vector.tensor_tensor(tmp[:], mean[:], mean[:], mybir.AluOpType.mult)
            nc.vector.tensor_tensor(var[:], ex2[:], tmp[:], mybir.AluOpType.subtract)
            nc.vector.tensor_scalar_add(var[:], var[:], 1e-5)
            nc.scalar.activation(std[:], var[:], mybir.ActivationFunctionType.Sqrt)
            nc.vector.reciprocal(rstd[:], std[:])
            nc.vector.tensor_tensor(scl[:], gamma[:], rstd[:], mybir.AluOpType.mult)
            nc.vector.tensor_tensor(tmp[:], mean[:], scl[:], mybir.AluOpType.mult)
            nc.vector.tensor_tensor(shf[:], beta[:], tmp[:], mybir.AluOpType.subtract)

            final = pp.tile([C, NSH], F32)
            nc.scalar.activation(final[:], pre[:, :NSH],
                                 mybir.ActivationFunctionType.Relu,
                                 bias=shf[:], scale=scl[:])
            nc.sync.dma_start(out_d[:], final[:])

    return nc


# ---------------------------------------------------------------------------
# Entry point
# ---------------------------------------------------------------------------

_CACHE = {}


def run_gcn(x, edge_index, W, gamma, beta, n_cores=8, split=32768,
            groups_per_chunk=4, trace=False):
    from concourse.bass_utils import run_bass_kernel_spmd

    st, in_maps = host_prep(x, edge_index, W, gamma, beta, n_cores, split,
                            groups_per_chunk)
    key = (st["N"], st["C"], tuple(st["tA"]), tuple(st["tB"]))
    nc = build_program(st)
    nc.compile()
    res = run_bass_kernel_spmd(nc, in_maps, list(range(n_cores)), trace=trace)
    NSH = st["NSH"]
    out = np.concatenate([np.asarray(res.results[c]["outT"], np.float32).T
                          for c in range(n_cores)], axis=0)
    return out, res


def kernel(x, edge_index, W, bias, gamma, beta):
    out, _ = run_gcn(np.asarray(x, np.float32), np.asarray(edge_index),
                     np.asarray(W, np.float32), np.asarray(gamma, np.float32),
                     np.asarray(beta, np.float32))
    return out
